# revision 1
# baseline (speedup 1.0000x reference)
"""LoKr linear forward on 8 TRN2 NeuronCores.

out = x @ (W0 + (alpha/lora_dim) * kron(w1, w2_a @ w2_b)).T + b

Strategy: fold the LoKr delta into the weight on host (O(16M) flops,
negligible vs the 550 GFLOP matmul), shard x over tokens data-parallel
across 8 cores, and run a blocked bf16 matmul per core:
  - xT shard [4096, 2048] bf16, W_effT [4096, 4096] bf16 (host-prepped
    transposed layouts so the contraction dim lands on SBUF partitions)
  - per core: loop t-halves (1024 tokens) x 8 o-blocks (512 outs);
    W o-block (32 k-tiles) double-buffered, psum accumulates over k,
    bias added on the DVE during PSUM->SBUF eviction, fp32 out.
"""
import sys, types

sys.path.insert(0, '/opt/trn_rl_repo')

import numpy as np
import ml_dtypes
import concourse.bass as bass
import concourse.mybir as mybir
import concourse.tile as tile
import concourse.bass_utils as bass_utils

ALPHA = 1.0
LORA_DIM = 4
MULTIPLIER = 1.0

N_CORES = 8
B, S, IN, OUT = 4, 4096, 4096, 4096
T_CORE = B * S // N_CORES          # 2048 tokens per core
T_HALF = T_CORE // 2               # 1024
KT = 128                           # contraction tile (SBUF partitions)
TT = 128                           # token tile (psum partitions)
OT = 512                           # out-feature tile (psum free dim)
NK = IN // KT                      # 32
NO = OUT // OT                     # 8
NT = T_HALF // TT                  # 8


def _split_multi_waits(nc):
    """This walrus build encodes at most ONE semaphore wait per ISA
    instruction; hoist extra waits onto single-wait NOPs inserted before."""
    ctr = 0
    for f in nc.m.functions:
        for blk in f.blocks:
            out = []
            changed = False
            for i in blk.instructions:
                si = i.sync_info
                if si is not None and si.on_wait and len(si.on_wait) > 1:
                    waits = list(si.on_wait)
                    for w in waits[:-1]:
                        ctr += 1
                        out.append(mybir.InstNoOp(
                            name=f"I-wsplit-{ctr}",
                            engine=i.engine, ins=[], outs=[],
                            sync_info=mybir.SyncInfo(on_wait=[w], on_update=[]),
                        ))
                    i.sync_info = mybir.SyncInfo(
                        on_wait=[waits[-1]], on_update=list(si.on_update))
                    changed = True
                out.append(i)
            if changed:
                blk.instructions = out


def build_nc():
    nc = bass.Bass(trn_type="TRN2")
    bf16 = mybir.dt.bfloat16
    f32 = mybir.dt.float32
    xT = nc.dram_tensor("xT", [IN, T_CORE], bf16, kind="ExternalInput")
    wT = nc.dram_tensor("wT", [IN, OUT], bf16, kind="ExternalInput")
    bias_d = nc.dram_tensor("bias", [128, OUT], f32, kind="ExternalInput")
    out = nc.dram_tensor("out", [T_CORE, OUT], f32, kind="ExternalOutput")

    with tile.TileContext(nc) as tc:
        with (
            tc.tile_pool(name="const", bufs=1) as constp,
            tc.tile_pool(name="xp", bufs=NK) as xp,
            tc.tile_pool(name="wp", bufs=2 * NK) as wp,
            tc.tile_pool(name="op", bufs=4) as op,
            tc.tile_pool(name="ps", bufs=4, space="PSUM") as pp,
        ):
            bias = constp.tile([128, OUT], f32)
            nc.sync.dma_start(bias[:], bias_d[:])

            for th in range(2):
                t0 = th * T_HALF
                # x half-shard resident: 32 tiles [128, 1024] bf16
                xts = []
                for k in range(NK):
                    xt = xp.tile([KT, T_HALF], bf16, tag="x")
                    nc.sync.dma_start(
                        xt[:], xT[k*KT:(k+1)*KT, t0:t0+T_HALF])
                    xts.append(xt)
                for o in range(NO):
                    wts = []
                    for k in range(NK):
                        wt = wp.tile([KT, OT], bf16, tag="w")
                        nc.sync.dma_start(
                            wt[:], wT[k*KT:(k+1)*KT, o*OT:(o+1)*OT])
                        wts.append(wt)
                    for tt in range(NT):
                        ps = pp.tile([TT, OT], f32)
                        for k in range(NK):
                            nc.tensor.matmul(
                                ps[:], xts[k][:, tt*TT:(tt+1)*TT], wts[k][:],
                                start=(k == 0), stop=(k == NK - 1))
                        ot = op.tile([TT, OT], f32)
                        nc.vector.tensor_add(
                            ot[:], ps[:], bias[:, o*OT:(o+1)*OT])
                        nc.sync.dma_start(
                            out[t0+tt*TT:t0+(tt+1)*TT, o*OT:(o+1)*OT], ot[:])
    _split_multi_waits(nc)
    return nc


_NC_CACHE = []


def _get_nc():
    if not _NC_CACHE:
        _NC_CACHE.append(build_nc())
    return _NC_CACHE[0]


def make_in_maps(x, W0, b, lokr_w1, lokr_w2_a, lokr_w2_b):
    scale = (ALPHA / LORA_DIM) * MULTIPLIER
    w2 = lokr_w2_a.astype(np.float32) @ lokr_w2_b.astype(np.float32)
    w_eff = W0.astype(np.float32) + scale * np.kron(
        lokr_w1.astype(np.float32), w2)
    wT_bf = np.ascontiguousarray(w_eff.T).astype(ml_dtypes.bfloat16)
    bias_rep = np.ascontiguousarray(
        np.broadcast_to(b.astype(np.float32)[None, :], (128, OUT)))
    xs = x.astype(np.float32).reshape(B * S, IN)
    in_maps = []
    for c in range(N_CORES):
        shard = xs[c*T_CORE:(c+1)*T_CORE]
        xT_bf = np.ascontiguousarray(shard.T).astype(ml_dtypes.bfloat16)
        in_maps.append({"xT": xT_bf, "wT": wT_bf, "bias": bias_rep})
    return in_maps


def run_spmd(in_maps, trace=False, **kw):
    nc = _get_nc()
    return bass_utils.run_bass_kernel_spmd(
        nc, in_maps, core_ids=list(range(N_CORES)), trace=trace, **kw)


def kernel(x, W0, b, lokr_w1, lokr_w2_a, lokr_w2_b):
    in_maps = make_in_maps(x, W0, b, lokr_w1, lokr_w2_a, lokr_w2_b)
    res = run_spmd(in_maps, trace=False)
    out = np.concatenate(
        [res.results[c]["out"] for c in range(N_CORES)], axis=0)
    return out.reshape(B, S, OUT).astype(np.float32)


# revision 3
# speedup vs baseline: 1.0118x; 1.0118x over previous
"""LoKr linear forward on 8 TRN2 NeuronCores.

out = x @ (W0 + (alpha/lora_dim) * kron(w1, w2_a @ w2_b)).T + b

Strategy: fold the LoKr delta into the weight on host (O(16M) flops,
negligible vs the 550 GFLOP matmul), shard x over tokens data-parallel
across 8 cores, and run a blocked bf16 matmul per core:
  - xT shard [4096, 2048] bf16, W_effT [4096, 4096] bf16 (host-prepped
    transposed layouts so the contraction dim lands on SBUF partitions)
  - per core: loop t-halves (1024 tokens) x 8 o-blocks (512 outs);
    W o-block (32 k-tiles) double-buffered, psum accumulates over k,
    bias added on the DVE during PSUM->SBUF eviction, fp32 out.
"""
import sys, types

sys.path.insert(0, '/opt/trn_rl_repo')

import numpy as np
import ml_dtypes
import concourse.bass as bass
import concourse.mybir as mybir
import concourse.tile as tile
import concourse.bass_utils as bass_utils

ALPHA = 1.0
LORA_DIM = 4
MULTIPLIER = 1.0

N_CORES = 8
B, S, IN, OUT = 4, 4096, 4096, 4096
T_CORE = B * S // N_CORES          # 2048 tokens per core
T_HALF = T_CORE // 2               # 1024
KT = 128                           # contraction tile (SBUF partitions)
TT = 128                           # token tile (psum partitions)
OT = 512                           # out-feature tile (psum free dim)
NK = IN // KT                      # 32
NO = OUT // OT                     # 8
NT = T_HALF // TT                  # 8


def _split_multi_waits(nc):
    """This walrus build encodes at most ONE semaphore wait per ISA
    instruction; hoist extra waits onto single-wait NOPs inserted before."""
    ctr = 0
    for f in nc.m.functions:
        for blk in f.blocks:
            out = []
            changed = False
            for i in blk.instructions:
                si = i.sync_info
                if si is not None and si.on_wait and len(si.on_wait) > 1:
                    waits = list(si.on_wait)
                    for w in waits[:-1]:
                        ctr += 1
                        out.append(mybir.InstNoOp(
                            name=f"I-wsplit-{ctr}",
                            engine=i.engine, ins=[], outs=[],
                            sync_info=mybir.SyncInfo(on_wait=[w], on_update=[]),
                        ))
                    i.sync_info = mybir.SyncInfo(
                        on_wait=[waits[-1]], on_update=list(si.on_update))
                    changed = True
                out.append(i)
            if changed:
                blk.instructions = out


def build_nc():
    nc = bass.Bass(trn_type="TRN2")
    bf16 = mybir.dt.bfloat16
    f32 = mybir.dt.float32
    xT = nc.dram_tensor("xT", [IN, T_CORE], bf16, kind="ExternalInput")
    wT = nc.dram_tensor("wT", [IN, OUT], bf16, kind="ExternalInput")
    bias_d = nc.dram_tensor("bias", [128, OUT], f32, kind="ExternalInput")
    out = nc.dram_tensor("out", [T_CORE, OUT], f32, kind="ExternalOutput")

    with tile.TileContext(nc) as tc:
        with (
            tc.tile_pool(name="const", bufs=1) as constp,
            tc.tile_pool(name="warm", bufs=1) as warmp,
            tc.tile_pool(name="xp", bufs=NK) as xp,
            tc.tile_pool(name="wp", bufs=2 * NK) as wp,
            tc.tile_pool(name="op", bufs=4) as op,
            tc.tile_pool(name="ps", bufs=4, space="PSUM") as pp,
            tc.tile_pool(name="wps", bufs=1, space="PSUM") as wpp,
        ):
            # PE warm-up: keep TensorE busy while the first DMAs land so the
            # HAM clock-gate opens (K=8/8) before real matmuls start.
            wz = warmp.tile([KT, OT], bf16)
            nc.gpsimd.memset(wz[:], 0.0)
            wps = wpp.tile([TT, OT], f32)
            for _ in range(64):
                nc.tensor.matmul(wps[:], wz[:, :TT], wz[:], start=True,
                                 stop=True)

            bias = constp.tile([128, OUT], f32)

            for th in range(2):
                t0 = th * T_HALF
                # x half-shard resident: 32 tiles [128, 1024] bf16.
                # Interleave with the first o-block's W tiles so the k=0,1,...
                # matmuls can start as soon as their pair arrives.
                xts = []
                w0ts = []
                for k in range(NK):
                    wt = wp.tile([KT, OT], bf16, tag="w")
                    nc.sync.dma_start(wt[:], wT[k*KT:(k+1)*KT, 0:OT])
                    w0ts.append(wt)
                    xt = xp.tile([KT, T_HALF], bf16, tag="x")
                    nc.sync.dma_start(
                        xt[:], xT[k*KT:(k+1)*KT, t0:t0+T_HALF])
                    xts.append(xt)
                if th == 0:
                    nc.sync.dma_start(bias[:], bias_d[:])
                for o in range(NO):
                    if o == 0:
                        wts = w0ts
                    else:
                        wts = []
                        for k in range(NK):
                            wt = wp.tile([KT, OT], bf16, tag="w")
                            nc.sync.dma_start(
                                wt[:], wT[k*KT:(k+1)*KT, o*OT:(o+1)*OT])
                            wts.append(wt)
                    for tt in range(NT):
                        ps = pp.tile([TT, OT], f32)
                        for k in range(NK):
                            nc.tensor.matmul(
                                ps[:], xts[k][:, tt*TT:(tt+1)*TT], wts[k][:],
                                start=(k == 0), stop=(k == NK - 1))
                        ot = op.tile([TT, OT], f32)
                        nc.vector.tensor_add(
                            ot[:], ps[:], bias[:, o*OT:(o+1)*OT])
                        nc.sync.dma_start(
                            out[t0+tt*TT:t0+(tt+1)*TT, o*OT:(o+1)*OT], ot[:])
    _split_multi_waits(nc)
    return nc


_NC_CACHE = []


def _get_nc():
    if not _NC_CACHE:
        _NC_CACHE.append(build_nc())
    return _NC_CACHE[0]


def make_in_maps(x, W0, b, lokr_w1, lokr_w2_a, lokr_w2_b):
    scale = (ALPHA / LORA_DIM) * MULTIPLIER
    w2 = lokr_w2_a.astype(np.float32) @ lokr_w2_b.astype(np.float32)
    w_eff = W0.astype(np.float32) + scale * np.kron(
        lokr_w1.astype(np.float32), w2)
    wT_bf = np.ascontiguousarray(w_eff.T).astype(ml_dtypes.bfloat16)
    bias_rep = np.ascontiguousarray(
        np.broadcast_to(b.astype(np.float32)[None, :], (128, OUT)))
    xs = x.astype(np.float32).reshape(B * S, IN)
    in_maps = []
    for c in range(N_CORES):
        shard = xs[c*T_CORE:(c+1)*T_CORE]
        xT_bf = np.ascontiguousarray(shard.T).astype(ml_dtypes.bfloat16)
        in_maps.append({"xT": xT_bf, "wT": wT_bf, "bias": bias_rep})
    return in_maps


def run_spmd(in_maps, trace=False, **kw):
    nc = _get_nc()
    return bass_utils.run_bass_kernel_spmd(
        nc, in_maps, core_ids=list(range(N_CORES)), trace=trace, **kw)


def kernel(x, W0, b, lokr_w1, lokr_w2_a, lokr_w2_b):
    in_maps = make_in_maps(x, W0, b, lokr_w1, lokr_w2_a, lokr_w2_b)
    res = run_spmd(in_maps, trace=False)
    out = np.concatenate(
        [res.results[c]["out"] for c in range(N_CORES)], axis=0)
    return out.reshape(B, S, OUT).astype(np.float32)


# revision 7
# speedup vs baseline: 1.0317x; 1.0197x over previous
"""LoKr linear forward on 8 TRN2 NeuronCores.

out = x @ (W0 + (alpha/lora_dim) * kron(w1, w2_a @ w2_b)).T + b

Strategy: fold the LoKr delta into the weight on host (O(16M) flops,
negligible vs the 550 GFLOP matmul), shard x over tokens data-parallel
across 8 cores, and run a blocked bf16 matmul per core:
  - xT shard [4096, 2048] bf16, W_effT [4096, 4096] bf16 (host-prepped
    transposed layouts so the contraction dim lands on SBUF partitions)
  - per core: loop t-halves (1024 tokens) x 8 o-blocks (512 outs);
    W o-block (32 k-tiles) double-buffered, psum accumulates over k,
    bias added on the DVE during PSUM->SBUF eviction, fp32 out.
"""
import sys, types

sys.path.insert(0, '/opt/trn_rl_repo')

import numpy as np
import ml_dtypes
import concourse.bass as bass
import concourse.mybir as mybir
import concourse.tile as tile
import concourse.bass_utils as bass_utils

ALPHA = 1.0
LORA_DIM = 4
MULTIPLIER = 1.0

N_CORES = 8
B, S, IN, OUT = 4, 4096, 4096, 4096
T_CORE = B * S // N_CORES          # 2048 tokens per core
T_HALF = T_CORE // 2               # 1024
KT = 128                           # contraction tile (SBUF partitions)
TT = 128                           # token tile (psum partitions)
OT = 512                           # out-feature tile (psum free dim)
NK = IN // KT                      # 32
NO = OUT // OT                     # 8
NT = T_HALF // TT                  # 8


def _split_multi_waits(nc):
    """This walrus build encodes at most ONE semaphore wait per ISA
    instruction; hoist extra waits onto single-wait NOPs inserted before."""
    ctr = 0
    for f in nc.m.functions:
        for blk in f.blocks:
            out = []
            changed = False
            for i in blk.instructions:
                si = i.sync_info
                if si is not None and si.on_wait and len(si.on_wait) > 1:
                    waits = list(si.on_wait)
                    for w in waits[:-1]:
                        ctr += 1
                        out.append(mybir.InstNoOp(
                            name=f"I-wsplit-{ctr}",
                            engine=i.engine, ins=[], outs=[],
                            sync_info=mybir.SyncInfo(on_wait=[w], on_update=[]),
                        ))
                    i.sync_info = mybir.SyncInfo(
                        on_wait=[waits[-1]], on_update=list(si.on_update))
                    changed = True
                out.append(i)
            if changed:
                blk.instructions = out


def build_nc():
    nc = bass.Bass(trn_type="TRN2")
    bf16 = mybir.dt.bfloat16
    f32 = mybir.dt.float32
    xT = nc.dram_tensor("xT", [IN, T_CORE], bf16, kind="ExternalInput")
    wT = nc.dram_tensor("wT", [IN, OUT], bf16, kind="ExternalInput")
    bias_d = nc.dram_tensor("bias", [128, OUT], f32, kind="ExternalInput")
    out = nc.dram_tensor("out", [T_CORE, OUT], f32, kind="ExternalOutput")

    with tile.TileContext(nc) as tc:
        with (
            tc.tile_pool(name="const", bufs=1) as constp,
            tc.tile_pool(name="warm", bufs=1) as warmp,
            tc.tile_pool(name="xp", bufs=NK) as xp,
            tc.tile_pool(name="wp", bufs=2 * NK) as wp,
            tc.tile_pool(name="op", bufs=8) as op,
            tc.tile_pool(name="ps", bufs=8, space="PSUM") as pp,
        ):
            # PE warm-up: keep TensorE busy while the first DMAs land so the
            # HAM clock-gate opens (K=8/8) before real matmuls start.
            wz = warmp.tile([KT, OT], bf16)
            nc.gpsimd.memset(wz[:], 0.0)
            wps = pp.tile([TT, OT], f32, tag="ps")
            for _ in range(96):
                nc.tensor.matmul(wps[:, :TT], wz[:, :TT], wz[:, :TT],
                                 start=True, stop=True)

            bias = constp.tile([128, OUT], f32)

            for th in range(2):
                t0 = th * T_HALF
                # x half-shard resident: 32 tiles [128, 1024] bf16.
                # Interleave with the first o-block's W tiles so the k=0,1,...
                # matmuls can start as soon as their pair arrives.
                xts = []
                w0ts = []
                for k in range(NK):
                    wt = wp.tile([KT, OT], bf16, tag="w")
                    nc.sync.dma_start(wt[:], wT[k*KT:(k+1)*KT, 0:OT])
                    w0ts.append(wt)
                    xt = xp.tile([KT, T_HALF], bf16, tag="x")
                    nc.sync.dma_start(
                        xt[:], xT[k*KT:(k+1)*KT, t0:t0+T_HALF])
                    xts.append(xt)
                if th == 0:
                    nc.sync.dma_start(bias[:], bias_d[:])
                for o in range(NO):
                    if o == 0:
                        # k-outer / t-inner across all 8 PSUM banks: the PE
                        # consumes each freshly-DMA'd (w, x) tile pair for all
                        # 8 token tiles at once, tracking the DMA wavefront
                        # instead of stalling on one accumulation chain.
                        wts = w0ts
                        pss = [pp.tile([TT, OT], f32, tag="ps",
                                        name=f"pss_{th}_{i}")
                               for i in range(NT)]
                        for k in range(NK):
                            for tt in range(NT):
                                nc.tensor.matmul(
                                    pss[tt][:],
                                    xts[k][:, tt*TT:(tt+1)*TT], wts[k][:],
                                    start=(k == 0), stop=(k == NK - 1))
                        for tt in range(NT):
                            ot = op.tile([TT, OT], f32)
                            nc.vector.tensor_add(
                                ot[:], pss[tt][:], bias[:, o*OT:(o+1)*OT])
                            nc.sync.dma_start(
                                out[t0+tt*TT:t0+(tt+1)*TT, o*OT:(o+1)*OT],
                                ot[:])
                        continue
                    wts = []
                    for k in range(NK):
                        wt = wp.tile([KT, OT], bf16, tag="w")
                        nc.sync.dma_start(
                            wt[:], wT[k*KT:(k+1)*KT, o*OT:(o+1)*OT])
                        wts.append(wt)
                    for tt in range(NT):
                        ps = pp.tile([TT, OT], f32, tag="ps")
                        for k in range(NK):
                            nc.tensor.matmul(
                                ps[:], xts[k][:, tt*TT:(tt+1)*TT], wts[k][:],
                                start=(k == 0), stop=(k == NK - 1))
                        ot = op.tile([TT, OT], f32)
                        nc.vector.tensor_add(
                            ot[:], ps[:], bias[:, o*OT:(o+1)*OT])
                        nc.sync.dma_start(
                            out[t0+tt*TT:t0+(tt+1)*TT, o*OT:(o+1)*OT], ot[:])
    _split_multi_waits(nc)
    return nc


_NC_CACHE = []


def _get_nc():
    if not _NC_CACHE:
        _NC_CACHE.append(build_nc())
    return _NC_CACHE[0]


def make_in_maps(x, W0, b, lokr_w1, lokr_w2_a, lokr_w2_b):
    scale = (ALPHA / LORA_DIM) * MULTIPLIER
    w2 = lokr_w2_a.astype(np.float32) @ lokr_w2_b.astype(np.float32)
    w_eff = W0.astype(np.float32) + scale * np.kron(
        lokr_w1.astype(np.float32), w2)
    wT_bf = np.ascontiguousarray(w_eff.T).astype(ml_dtypes.bfloat16)
    bias_rep = np.ascontiguousarray(
        np.broadcast_to(b.astype(np.float32)[None, :], (128, OUT)))
    xs = x.astype(np.float32).reshape(B * S, IN)
    in_maps = []
    for c in range(N_CORES):
        shard = xs[c*T_CORE:(c+1)*T_CORE]
        xT_bf = np.ascontiguousarray(shard.T).astype(ml_dtypes.bfloat16)
        in_maps.append({"xT": xT_bf, "wT": wT_bf, "bias": bias_rep})
    return in_maps


def run_spmd(in_maps, trace=False, **kw):
    nc = _get_nc()
    return bass_utils.run_bass_kernel_spmd(
        nc, in_maps, core_ids=list(range(N_CORES)), trace=trace, **kw)


def kernel(x, W0, b, lokr_w1, lokr_w2_a, lokr_w2_b):
    in_maps = make_in_maps(x, W0, b, lokr_w1, lokr_w2_a, lokr_w2_b)
    res = run_spmd(in_maps, trace=False)
    out = np.concatenate(
        [res.results[c]["out"] for c in range(N_CORES)], axis=0)
    return out.reshape(B, S, OUT).astype(np.float32)
